# revision 22
# baseline (speedup 1.0000x reference)
"""Trainium2 Bass kernel for nn_Discriminator: 5-layer GRU stack + output projection.

Strategy (V2)
-------------
Pure data parallel over batch: 1024 batch rows -> 8 cores x 128. Each core runs
the full T=512 sequential scan on its shard.

Mapping (per core):
- Feature-major layout: every on-chip tensor is [feature_partitions, batch=128].
- Layer wavefront: at tick tau, layer l processes timestep (tau - l). All five
  layers' gate math is packed into shared instructions across partitions
  (state rows: l0 0:32, l1 32:48, l2 48:56, l3 56:72, l4 72:104).
- Sigmoid-space state: g = (h+1)/2, so tanh(x) = 2*sigmoid(2x)-1 folds into the
  same sigmoid ACT table, and the update stays a lerp: g' = zc*m + (1-zc)*g
  with zc = 1-z (computed directly by negating the z-gate weights).
- fp16 everywhere off-psum: all matmul inputs (weights, states, x) are fp16
  (1 cyc/col on PE vs 4 for fp32), elementwise DVE ops run on fp16 SBUF
  tensors (4x mode). PSUM accumulation stays fp32.
- Split-state trick: instead of materializing g' = zc*m + (1-zc)*g on the
  critical path, carry u = zc*m and wt = (zc-1)*g separately and compute
  p = W@u + (-W)@wt as two matmul accumulations. The wt half runs mid-tick
  (off-path); only the u-matmuls gate the next tick.
- One PSUM bank per tick holds all four gate pre-activations as column
  groups [104, pR|pZ|pN|pHN] so x/wt/u matmuls accumulate region-wise.
- Pool engine copies pN/pHN psum->fp16 SBUF in the shadow of sigma_r so the
  critical DVE ops (tm = r*pHN, v = tm + pN) run in 4x half-word mode.
- Y: two matmuls per tick (WY@u - WY@wt) into a [64, 4*128] psum bank,
  DMA'd straight from PSUM to HBM every 4 ticks.
- Input X is transposed host-side to [T, 64, B] fp16 so per-tick x_t tiles are
  direct DMA loads; output is produced as [T, 64, B] fp32 and untransposed
  host-side.
"""

import numpy as np

D = 64
T_FULL = 512
BZ = 1024
NCORES = 8
BC = BZ // NCORES  # 128 batch per core
H = [32, 16, 8, 16, 32]
OFFS = [0, 32, 48, 56, 72]  # state row offset per layer
SH = 104  # sum(H)
SA = 105  # state rows + ones row


def _build_weights(inp):
    """Fold the reference GRU weights into the packed fp16 kernel matrices.

    Kernel math per layer l at one tick (all in sigmoid space g=(h+1)/2):
      pR = WR.T @ g_aug (+ Wir0 @ x for l=0 rows)        ; r  = sigmoid(pR)
      pZ = WZ.T @ g_aug (+ -Wiz0 @ x for l=0 rows)       ; zc = sigmoid(pZ) = 1-z
      pN = WN.T @ g_aug (+ 2*Win0 @ x for l=0 rows)      ; (= 2*inn + 2*b_in)
      pHN = WHN.T @ g_aug                                ; (= 2*hn)
      m  = sigmoid(pN + r*pHN)   (= (tanh(nin)+1)/2)
      g' = zc*m + (1-zc)*g  carried as u = zc*m, wt = (zc-1)*g, g' = u - wt
    """
    f32 = np.float32
    f16 = np.float16
    WR = np.zeros((SA, SH), f32)
    WZ = np.zeros((SA, SH), f32)
    WN = np.zeros((SA, SH), f32)
    WHN = np.zeros((SA, SH), f32)
    # x-side weights, zero-padded to the full 104 gate rows so the hoisted
    # x-matmuls initialize the whole psum region (clean has_written bits)
    W0 = np.zeros((D, 3 * SH), f32)
    for l in range(5):
        dh = H[l]
        o = OFFS[l]
        w_ih = np.asarray(inp[f"w_ih_{l}"], f32)  # [3H, di]
        w_hh = np.asarray(inp[f"w_hh_{l}"], f32)  # [3H, dh]
        b_ih = np.asarray(inp[f"b_ih_{l}"], f32)
        b_hh = np.asarray(inp[f"b_hh_{l}"], f32)
        Wir, Wiz, Win = w_ih[:dh], w_ih[dh : 2 * dh], w_ih[2 * dh :]
        Whr, Whz, Whn = w_hh[:dh], w_hh[dh : 2 * dh], w_hh[2 * dh :]
        bir, biz, bin_ = b_ih[:dh], b_ih[dh : 2 * dh], b_ih[2 * dh :]
        bhr, bhz, bhn = b_hh[:dh], b_hh[dh : 2 * dh], b_hh[2 * dh :]

        # recurrent (own-state) parts: h = 2g-1 -> W@h = (2W)@g - rowsum(W)
        WR[o : o + dh, o : o + dh] = (2.0 * Whr).T
        WZ[o : o + dh, o : o + dh] = -(2.0 * Whz).T
        WHN[o : o + dh, o : o + dh] = (4.0 * Whn).T
        r_bias = bir + bhr - Whr.sum(1)
        z_bias = biz + bhz - Whz.sum(1)
        hn_bias = 2.0 * (bhn - Whn.sum(1))
        n_bias = 2.0 * bin_

        if l == 0:
            # x enters raw through W0 (three SH-col blocks: r | z(neg) | n(x2))
            W0[:, 0:32] = Wir.T
            W0[:, SH : SH + 32] = -Wiz.T
            W0[:, 2 * SH : 2 * SH + 32] = (2.0 * Win).T
        else:
            po, pd = OFFS[l - 1], H[l - 1]
            WR[po : po + pd, o : o + dh] = (2.0 * Wir).T
            WZ[po : po + pd, o : o + dh] = -(2.0 * Wiz).T
            WN[po : po + pd, o : o + dh] = (4.0 * Win).T
            r_bias = r_bias - Wir.sum(1)
            z_bias = z_bias - Wiz.sum(1)
            n_bias = n_bias - 2.0 * Win.sum(1)

        WR[SH, o : o + dh] = r_bias
        WZ[SH, o : o + dh] = -z_bias
        WN[SH, o : o + dh] = n_bias
        WHN[SH, o : o + dh] = hn_bias

    w_out = np.asarray(inp["w_out"], f32)  # [64, 32]
    b_out = np.asarray(inp["b_out"], f32)  # [64]
    WY = np.zeros((SA, D), f32)
    WY[OFFS[4] : OFFS[4] + 32, :] = (2.0 * w_out).T
    WY[SH, :] = b_out - w_out.sum(1)

    W = {}
    for nm, mat in (("WR", WR), ("WZ", WZ), ("WN", WN), ("WHN", WHN), ("WY", WY)):
        m16 = mat.astype(f16)
        W[nm] = m16
        W[nm + "N"] = -m16
    W["W0"] = W0.astype(f16)

    # init/reset constants (DMA'd into SBUF; engine APs need 32-aligned
    # partition starts, DMA writes don't)
    UINIT = np.full((SA, BC), 0.5, f16)
    UINIT[SH] = 1.0
    WTINIT = np.zeros((SA, BC), f16)
    GINIT = np.full((SA, BC), 0.5, f16)
    GINIT[SH] = 1.0
    W["UINIT"], W["WTINIT"], W["GINIT"] = UINIT, WTINIT, GINIT
    return W


def numpy_forward(inputs, T):
    """Numpy model of the exact kernel math incl. fp16 casts (for validation)."""
    W = _build_weights(inputs)
    f32 = np.float32
    q = lambda a: np.asarray(a, np.float16).astype(f32)
    WR, WZ, WN, WHN = (W[k].astype(f32) for k in ("WR", "WZ", "WN", "WHN"))
    W0, WY = W["W0"].astype(f32), W["WY"].astype(f32)
    X = np.asarray(inputs["imputed_X"], f32)
    B = X.shape[0]
    X = X[:, :T]
    sig = lambda x: 1.0 / (1.0 + np.exp(-x))
    u = np.full((SA, B), 0.5, f32)
    wt = np.zeros((SA, B), f32)
    g = np.full((SA, B), 0.5, f32)
    u[SH] = 1.0
    g[SH] = 1.0
    Y = np.zeros((T, D, B), f32)
    for tau in range(T + 4):
        pR = WR.T @ u - WR.T @ wt
        pZ = WZ.T @ u - WZ.T @ wt
        pN = WN.T @ u - WN.T @ wt
        pHN = WHN.T @ u - WHN.T @ wt
        if tau < T:
            x = q(X[:, tau, :].T)  # [64, B]
            pR += W0[:, 0:SH].T @ x
            pZ += W0[:, SH : 2 * SH].T @ x
            pN += W0[:, 2 * SH : 3 * SH].T @ x
        r = q(sig(pR))
        zc = q(sig(pZ))
        tm = q(r * q(pHN))
        v = q(tm + q(pN))
        m = q(sig(v))
        u_new = q(zc * m)
        wt_new = q((zc - 1.0) * g[:SH])
        g_new = u_new - wt_new
        u = np.concatenate([u_new, np.ones((1, B), f32)], 0)
        wt = np.concatenate([wt_new, np.zeros((1, B), f32)], 0)
        g = np.concatenate([g_new, np.ones((1, B), f32)], 0)
        for l in range(1, 5):
            if tau == l - 1:
                u[OFFS[l] : OFFS[l] + H[l]] = 0.5
                wt[OFFS[l] : OFFS[l] + H[l]] = 0.0
                g[OFFS[l] : OFFS[l] + H[l]] = 0.5
        if tau >= 4:
            Y[tau - 4] = WY.T @ u - WY.T @ wt
    return Y.transpose(2, 0, 1)  # [B, T, 64]


_prog_cache = {}


def _split_excess_waits(nc, limit=1):
    """The walrus build here accepts at most one sync-wait per instruction;
    Tile emits several on barrier drains etc. Split extras onto NoOps.

    Wait ORDER matters for latency: leading NoOps block the engine SEQ until
    their sem fires, then each costs a SEQ decode slot before the real
    instruction dispatches. So put stale waits (producer far in the past) on
    the NoOps — they resolve instantly while the SEQ would be idle anyway —
    and keep the most-recently-produced (critical) sem on the instruction,
    which then parks in the wait queue and dispatches straight off the sem.
    Staleness = current emitted update-count of that sem minus the wait value.
    """
    from concourse import mybir

    n_new = 0
    for f in nc.m.functions:
        for bb in f.blocks:
            # running per-sem update counts in final schedule order
            sem_count = {}
            changed = False
            new_list = []
            for ins in bb.instructions:
                si = ins.sync_info
                if si is not None and si.on_wait and len(si.on_wait) > limit:
                    waits = list(si.on_wait)
                    # most stale first (largest count-surplus), critical last
                    waits.sort(
                        key=lambda x: -(
                            sem_count.get(x.ant_name or x.id, 0)
                            - (x.wait_value or 0)
                        )
                    )
                    while len(waits) > limit:
                        chunk, waits = waits[:limit], waits[limit:]
                        nop = mybir.InstNoOp(
                            name=f"{ins.name}-ws{n_new}",
                            engine=ins.engine,
                            sync_info=mybir.SyncInfo(on_wait=chunk, on_update=[]),
                        )
                        new_list.append(nop)
                        n_new += 1
                    ins.sync_info = mybir.SyncInfo(
                        on_wait=list(waits), on_update=list(si.on_update)
                    )
                    changed = True
                if si is not None:
                    for upd in si.on_update:
                        k = upd.ant_name or upd.id
                        sem_count[k] = sem_count.get(k, 0) + (upd.update_value or 1)
                new_list.append(ins)
            if changed:
                bb.instructions = new_list
    return n_new


def _build_program(T, reps=1):
    key = (T, reps)
    if key in _prog_cache:
        return _prog_cache[key]
    import concourse.bass as bass
    import concourse.tile as tile
    from concourse.tile import add_dep_helper
    from concourse import mybir

    f32 = mybir.dt.float32
    f16 = mybir.dt.float16
    SIG = mybir.ActivationFunctionType.Sigmoid
    SUB = mybir.AluOpType.subtract
    MUL = mybir.AluOpType.mult

    XC = 32 if T % 32 == 0 else T  # ticks per input chunk
    NT = T + 4

    nc = bass.Bass(trn_type="TRN2", name=f"gru_v2_{T}_{reps}")
    XT = nc.dram_tensor("XT", [D, T, BC], f16, kind="ExternalInput")
    dW = {}
    for gk in ("WR", "WZ", "WN", "WHN"):
        dW[gk] = nc.dram_tensor(gk, [SA, SH], f16, kind="ExternalInput")
        dW[gk + "N"] = nc.dram_tensor(gk + "N", [SA, SH], f16, kind="ExternalInput")
    dW0 = nc.dram_tensor("W0", [D, 3 * SH], f16, kind="ExternalInput")
    dWY = nc.dram_tensor("WY", [SA, D], f16, kind="ExternalInput")
    dWYN = nc.dram_tensor("WYN", [SA, D], f16, kind="ExternalInput")
    dUINIT = nc.dram_tensor("UINIT", [SA, BC], f16, kind="ExternalInput")
    dWTINIT = nc.dram_tensor("WTINIT", [SA, BC], f16, kind="ExternalInput")
    dGINIT = nc.dram_tensor("GINIT", [SA, BC], f16, kind="ExternalInput")
    YT = nc.dram_tensor("YT", [D, T, BC], f32, kind="ExternalOutput")

    with tile.TileContext(nc) as tc:
        with (
            tc.tile_pool(name="consts", bufs=1) as consts,
            tc.tile_pool(name="upool", bufs=3) as upool,
            tc.tile_pool(name="wtpool", bufs=3) as wtpool,
            tc.tile_pool(name="gpool", bufs=3) as gpool,
            tc.tile_pool(name="xpool", bufs=2) as xpool,
            tc.tile_pool(name="work", bufs=2) as work,
            tc.tile_pool(name="ybufpool", bufs=2) as ybufpool,
            tc.tile_pool(name="psum", bufs=1, space="PSUM") as psum,
            tc.tile_pool(name="ypsum", bufs=2, space="PSUM") as ypsum,
        ):
            w = {}
            for gk in ("WR", "WZ", "WN", "WHN"):
                w[gk] = consts.tile([SA, SH], f16, tag=gk.lower(), name=gk.lower())
                w[gk + "N"] = consts.tile([SA, SH], f16, tag=gk.lower() + "n", name=gk.lower() + "n")
                nc.sync.dma_start(out=w[gk][:], in_=dW[gk][:])
                nc.sync.dma_start(out=w[gk + "N"][:], in_=dW[gk + "N"][:])
            w0 = consts.tile([D, 3 * SH], f16, tag="w0")
            wy = consts.tile([SA, D], f16, tag="wy")
            wyn = consts.tile([SA, D], f16, tag="wyn")
            uinit = consts.tile([SA, BC], f16, tag="uinit")
            wtinit = consts.tile([SA, BC], f16, tag="wtinit")
            ginit = consts.tile([SA, BC], f16, tag="ginit")
            for sb, dr in ((w0, dW0), (wy, dWY), (wyn, dWYN), (uinit, dUINIT),
                           (wtinit, dWTINIT), (ginit, dGINIT)):
                nc.sync.dma_start(out=sb[:], in_=dr[:])

            # pre-set the constant rows (104) of the rotating state tiles:
            # u row104 = 1, wt row104 = 0, g row104 = 1. DVE writes only rows
            # 0:104, so these survive the whole scan.
            state_bufs = []
            for pool, init in ((upool, dUINIT), (wtpool, dWTINIT), (gpool, dGINIT)):
                bufs = []
                for _ in range(3):
                    t_ = pool.tile([SA, BC], f16, tag="st", name="st")
                    nc.sync.dma_start(out=t_[:], in_=init[:])
                    bufs.append(t_)
                state_bufs.append(bufs)

            nchunks = (T + XC - 1) // XC
            xchunks = []

            def load_xchunk(ci):
                t0 = ci * XC
                nt = min(XC, T - t0)
                xc = xpool.tile([D, XC, BC], f16, tag="xc", name="xc")
                nc.sync.dma_start(out=xc[:, 0:nt, :], in_=XT[:, t0 : t0 + nt, :])
                xchunks.append(xc)

            for _rep in range(reps):
                del xchunks[:]
                load_xchunk(0)

                prev_u, prev_wt, prev_g = uinit, wtinit, ginit

                def alloc_bank(tau):
                    """New per-gate psum tiles for tick tau; run its x-side
                    matmuls (they only need the prefetched x chunk). Four
                    separate tiles: cross-engine readers of one psum tile
                    serialize in the dependency model."""
                    pR = psum.tile([SH, BC], f32, tag="pR", name="pR")
                    pZ = psum.tile([SH, BC], f32, tag="pZ", name="pZ")
                    pN = psum.tile([SH, BC], f32, tag="pN", name="pN")
                    pHN = psum.tile([SH, BC], f32, tag="pHN", name="pHN")
                    if tau < T:
                        xi = xchunks[tau // XC][:, tau % XC, :]
                        nc.tensor.matmul(pR[:], w0[:, 0:SH], xi, start=True,
                                         stop=False)
                        nc.tensor.matmul(pZ[:], w0[:, SH : 2 * SH], xi,
                                         start=True, stop=False)
                        nc.tensor.matmul(pN[:], w0[:, 2 * SH : 3 * SH], xi,
                                         start=True, stop=False)
                        has_x = True
                    else:
                        has_x = False
                    return (pR, pZ, pN, pHN), has_x

                def emit_wt_mms(bank, has_x, wt_t):
                    # recurrent wt-side accumulations for the NEXT tick's bank;
                    # the pHN tile's first writer is the HN wt-matmul
                    pR, pZ, pN, pHN = bank
                    nc.tensor.matmul(pHN[:], w["WHNN"][:], wt_t[:], start=True,
                                     stop=False)
                    nc.tensor.matmul(pR[:], w["WRN"][:], wt_t[:],
                                     start=not has_x, stop=False)
                    nc.tensor.matmul(pZ[:], w["WZN"][:], wt_t[:],
                                     start=not has_x, stop=False)
                    nc.tensor.matmul(pN[:], w["WNN"][:], wt_t[:],
                                     start=not has_x, stop=False)

                bank0, has_x0 = alloc_bank(0)
                emit_wt_mms(bank0, has_x0, wtinit)
                gates = (bank0, has_x0)

                ybank = None
                ybuf = None
                ybank_t0 = 0
                pending_ycopy = None
                for tau in range(NT + 1):
                    if tau < NT:
                        bank, has_x = gates
                        pR, pZ, pN, pHN = bank
                        if tau % XC == 0 and (tau // XC) + 1 < nchunks:
                            load_xchunk(tau // XC + 1)

                        # ---- critical-path u-matmuls: R first (gates sigma_r)
                        nc.tensor.matmul(pR[:], w["WR"][:], prev_u[:],
                                         start=False, stop=True)
                        nc.tensor.matmul(pHN[:], w["WHN"][:], prev_u[:],
                                         start=False, stop=True)
                        nc.tensor.matmul(pN[:], w["WN"][:], prev_u[:],
                                         start=False, stop=True)
                        nc.tensor.matmul(pZ[:], w["WZ"][:], prev_u[:],
                                         start=False, stop=True)

                    # ---- y projection for g(tau-1) = u(tau-1) - wt(tau-1)
                    if tau >= 5:
                        ty = tau - 5  # timestep index
                        cg = ty % 4
                        if cg == 0:
                            ybank = ypsum.tile([D, 4 * BC], f32, tag="yb", name="yb")
                            ybuf = ybufpool.tile([D, 4 * BC], f32, tag="ybuf",
                                                 name="ybuf")
                            ybank_t0 = ty
                        yseg = ybank[:, cg * BC : (cg + 1) * BC]
                        nc.tensor.matmul(yseg, wyn[:], prev_wt[:], start=True,
                                         stop=False)
                        nc.tensor.matmul(yseg, wy[:], prev_u[:], start=False,
                                         stop=True)
                        # psum is not DMA-visible: bounce each tick's column
                        # group through SBUF on the Pool engine (deferred to
                        # the end of the tick so it queues behind the critical
                        # phn/pn copies), DMA once 4 groups are staged
                        pending_ycopy = (ybank, ybuf, cg, ybank_t0)

                    if tau >= NT:
                        if pending_ycopy is not None:
                            yb_, ybf_, cg_, t0_ = pending_ycopy
                            nc.vector.tensor_copy(
                                ybf_[:, cg_ * BC : (cg_ + 1) * BC],
                                yb_[:, cg_ * BC : (cg_ + 1) * BC],
                            )
                            nc.sync.dma_start(
                                out=YT[:, t0_ : t0_ + cg_ + 1, :],
                                in_=ybf_[:, 0 : (cg_ + 1) * BC].rearrange(
                                    "p (t b) -> p t b", b=BC
                                ),
                            )
                            pending_ycopy = None
                        break

                    # ---- next tick's bank + x matmuls (PE idle window)
                    if tau + 1 <= NT - 1:
                        gates = alloc_bank(tau + 1)
                    else:
                        gates = None

                    # ---- activations and elementwise chain
                    r16 = work.tile([SH, BC], f16, tag="r")
                    zc16 = work.tile([SH, BC], f16, tag="zc")
                    nc.scalar.activation(r16[:], pR[:], SIG)
                    nc.scalar.activation(zc16[:], pZ[:], SIG)

                    # GPSIMD cannot touch PSUM on hw, so the psum->fp16-sbuf
                    # bounces run on DVE in the sigma_r shadow (their PE waits
                    # resolve while sigma_r is still running)
                    phn16 = work.tile([SH, BC], f16, tag="phn")
                    pn16 = work.tile([SH, BC], f16, tag="pn")
                    nc.vector.tensor_copy(phn16[:], pHN[:])
                    nc.vector.tensor_copy(pn16[:], pN[:])

                    tm = work.tile([SH, BC], f16, tag="tm")
                    v16 = work.tile([SH, BC], f16, tag="v")
                    tm_i = nc.vector.tensor_mul(tm[:], r16[:], phn16[:])
                    v_i = nc.vector.tensor_add(v16[:], tm[:], pn16[:])

                    # For warmup ticks, only layers 0..tau have valid state:
                    # write just their rows and leave deeper layers at the
                    # DMA-initialized values (u=0.5, wt=0, g=0.5), which is
                    # exactly the reset the wavefront needs — no reset DMAs.
                    # State buffers cycle every 3 ticks, so rows >= ke are
                    # still pristine from init for tau < 4 (first reuse is
                    # tau=3 of the tau=0 buffer, whose write stopped at 32).
                    ke = OFFS[tau + 1] if tau < 4 else SH

                    # wt(tau) = (zc - 1) * g(tau-1)   [off-path, feeds next bank]
                    wt_t = state_bufs[1][tau % 3]
                    stt_i = nc.vector.scalar_tensor_tensor(
                        wt_t[0:ke, :], zc16[0:ke, :], 1.0, prev_g[0:ke, :], SUB, MUL
                    )
                    # keep the DVE stream in path order: tm, v ahead of the
                    # off-path wt computation (the scheduler otherwise hoists
                    # wt-STT, serializing tm behind sigma_zc)
                    add_dep_helper(stt_i.ins, v_i.ins, sync=False,
                                   reason="dve path order")
                    if gates is not None:
                        emit_wt_mms(gates[0], gates[1], wt_t)

                    m16 = work.tile([SH, BC], f16, tag="m")
                    nc.scalar.activation(m16[:], v16[:], SIG)

                    # ---- state update: u = zc*m (path), g = u - wt (shadow)
                    u_t = state_bufs[0][tau % 3]
                    nc.vector.tensor_mul(u_t[0:ke, :], zc16[0:ke, :], m16[0:ke, :])
                    g_t = state_bufs[2][tau % 3]
                    g_i = nc.vector.tensor_sub(g_t[0:ke, :], u_t[0:ke, :],
                                               wt_t[0:ke, :])

                    if pending_ycopy is not None:
                        yb_, ybf_, cg_, t0_ = pending_ycopy
                        yc_i = nc.vector.tensor_copy(
                            ybf_[:, cg_ * BC : (cg_ + 1) * BC],
                            yb_[:, cg_ * BC : (cg_ + 1) * BC],
                        )
                        add_dep_helper(yc_i.ins, g_i.ins, sync=False,
                                       reason="y-copy off path")
                        if cg_ == 3:
                            nc.sync.dma_start(
                                out=YT[:, t0_ : t0_ + 4, :],
                                in_=ybf_[:].rearrange("p (t b) -> p t b", b=BC),
                            )
                        pending_ycopy = None

                    prev_u, prev_wt, prev_g = u_t, wt_t, g_t

    _split_excess_waits(nc)
    _prog_cache[key] = nc
    return nc


def _run(X_full, weights, T):
    """X_full: [BZ, T, D] float32. Returns [BZ, T, D]."""
    from concourse.bass_utils import run_bass_kernel_spmd

    nc = _build_program(T)
    in_maps = []
    for c in range(NCORES):
        xs = X_full[c * BC : (c + 1) * BC]  # [BC, T, D]
        XTc = np.ascontiguousarray(xs.transpose(2, 1, 0).astype(np.float16))
        in_maps.append({"XT": XTc, **weights})
    res = run_bass_kernel_spmd(nc, in_maps, core_ids=list(range(NCORES)))
    outs = []
    for c in range(NCORES):
        YTc = res.results[c]["YT"]  # [T, D, BC]
        outs.append(np.ascontiguousarray(YTc.transpose(2, 1, 0)))
    return np.concatenate(outs, 0)


def kernel(**inputs):
    X = np.asarray(inputs["imputed_X"], np.float32)
    weights = _build_weights(inputs)
    return _run(X, weights, X.shape[1])
